# revision 1
# baseline (speedup 1.0000x reference)
"""GCN+ReLU 2-layer kernel for Trainium2, 8 NeuronCores.

Strategy (dst-partitioned graph, per the sharding hint):
  - Nodes are split into 8 contiguous slices; each core owns the edges whose
    dst lands in its slice (host groups+sorts edges by dst once, in numpy).
  - segment_sum per 128-dst tile: gather the src feature rows with the custom
    dma_gather instruction (4 SWDGE queues, ~200GB/s of random 512B rows),
    then accumulate X^T @ S into PSUM where S is the one-hot dst-selection
    matrix built on-device (iota + is_equal). Aggregation runs on the *input*
    features (linearity: segment_sum(hW) = segment_sum(h) W), so the dense W
    matmul runs once per 128-dst tile, not per edge.
  - dma_gather takes int16 indices, so the gather source is split in 4 banks
    of <=32k rows; edges are grouped per (dst-tile, bank). Both layers share
    the same indices because the AllGather output is stored in true node
    order.
  - Layer outputs live in SBUF transposed [feat, node] so BatchNorm scale/
    shift are per-partition ops; global BN stats via a [128,2] AllReduce.
  - Between layers the normalized activations are written row-major and
    AllGather'd so every core can gather any src row in layer 2.
  - Uniform SPMD program: every core runs the identical instruction stream;
    per-core data (edge indices, one-hot ids) comes in as inputs. Per
    (dst-tile, bank) edge counts are padded across cores to a common block
    count (pad slots gather row 0 of the bank; their one-hot id is -1 so the
    selection matrix kills their contribution exactly).
"""
import sys
sys.path.insert(0, '/opt/trn_rl_repo')

from contextlib import ExitStack

import numpy as np

import concourse.bass as bass
import concourse.bacc as bacc_mod
import concourse.mybir as mybir
from concourse import bass_utils
from concourse.tile import TileContext

P = 128
D = 128
N_CORES = 8
N_BANKS = 4
BN_EPS = 1e-5

F32 = mybir.dt.float32
I32 = mybir.dt.int32
I16 = mybir.dt.int16
Alu = mybir.AluOpType
Act = mybir.ActivationFunctionType

# Ablation flags for debugging (set via kernel.ABLATE before _run)
ABLATE = set()


def _preprocess(src, dst, N, n_cores):
    """Group edges by (dst slice, dst tile, src bank); pad per (tile, bank)
    to a cross-core-uniform block count.

    Returns per-core idx16 [128, TOTCOLS] int16 and oh [128, TOTBLK] int32
    arrays plus the shared block structure nblk [T_NODE][N_BANKS].
    """
    NPC = N // n_cores
    T_NODE = -(-NPC // P)
    bank_rows = -(-N // N_BANKS)

    order = np.argsort(dst, kind="stable")
    src_s = src[order].astype(np.int64)
    dst_s = dst[order].astype(np.int64)

    core_lo = np.searchsorted(dst_s, np.arange(n_cores) * NPC)
    core_hi = np.searchsorted(dst_s, (np.arange(n_cores) + 1) * NPC)

    per = [[None] * T_NODE for _ in range(n_cores)]
    for c in range(n_cores):
        s_c = src_s[core_lo[c]:core_hi[c]]
        dl_c = dst_s[core_lo[c]:core_hi[c]] - c * NPC
        t_lo = np.searchsorted(dl_c, np.arange(T_NODE) * P)
        t_hi = np.searchsorted(dl_c, (np.arange(T_NODE) + 1) * P)
        for t in range(T_NODE):
            s_t = s_c[t_lo[t]:t_hi[t]]
            d_t = dl_c[t_lo[t]:t_hi[t]] - t * P
            b_t = s_t // bank_rows
            o = np.argsort(b_t, kind="stable")
            s_t, d_t, b_t = s_t[o], d_t[o], b_t[o]
            lo = np.searchsorted(b_t, np.arange(N_BANKS))
            hi = np.searchsorted(b_t, np.arange(N_BANKS) + 1)
            per[c][t] = [(s_t[lo[b]:hi[b]] - b * bank_rows,
                          d_t[lo[b]:hi[b]]) for b in range(N_BANKS)]

    nblk = [[0] * N_BANKS for _ in range(T_NODE)]
    for t in range(T_NODE):
        for b in range(N_BANKS):
            m = max(len(per[c][t][b][0]) for c in range(n_cores))
            nblk[t][b] = max(1, -(-m // P))

    totblk = sum(sum(r) for r in nblk)
    totcols = totblk * 8  # NI/16 idx columns per 128-slot block

    idx16_l, oh_l = [], []
    for c in range(n_cores):
        idx16 = np.zeros((P, totcols), np.int16)
        oh = np.full((P, totblk), -1, np.int32)
        blk0 = 0
        for t in range(T_NODE):
            for b in range(N_BANKS):
                nb = nblk[t][b]
                ni = nb * P
                s_tb, d_tb = per[c][t][b]
                arr = np.zeros(ni, np.int64)
                arr[:len(s_tb)] = s_tb
                # index i -> partition i%16 (replicated x8), col i//16
                tile16 = arr.reshape(ni // 16, 16).T.astype(np.int16)
                idx16[:, blk0 * 8:blk0 * 8 + nb * 8] = np.tile(tile16, (8, 1))
                ohv = np.full(ni, -1, np.int64)
                ohv[:len(d_tb)] = d_tb
                oh[:, blk0:blk0 + nb] = ohv.reshape(nb, P).T
                blk0 += nb
        idx16_l.append(idx16)
        oh_l.append(oh)

    meta = dict(NPC=NPC, T_NODE=T_NODE, bank_rows=bank_rows,
                totblk=totblk, totcols=totcols)
    return idx16_l, oh_l, nblk, meta


def _build(N, nblk, n_cores):
    NPC = N // n_cores
    T_NODE = -(-NPC // P)
    NPC_PAD = T_NODE * P
    bank_rows = -(-N // N_BANKS)
    totblk = sum(sum(r) for r in nblk)
    totcols = totblk * 8
    tbmax = max(sum(r) for r in nblk)
    groups = [list(range(n_cores))]
    n_last = NPC - (T_NODE - 1) * P

    nc = bacc_mod.Bacc(num_devices=n_cores, num_swdge_queues=4)

    hg = nc.dram_tensor("hg", [N, D], F32, kind="ExternalInput")
    hs = nc.dram_tensor("hs", [NPC_PAD, D], F32, kind="ExternalInput")
    i16d = nc.dram_tensor("i16", [P, totcols], I16, kind="ExternalInput")
    ohd = nc.dram_tensor("oh", [P, totblk], I32, kind="ExternalInput")
    w0d = nc.dram_tensor("w0", [D, D], F32, kind="ExternalInput")
    wr0d = nc.dram_tensor("wr0", [D, D], F32, kind="ExternalInput")
    w1d = nc.dram_tensor("w1", [D, D], F32, kind="ExternalInput")
    wr1d = nc.dram_tensor("wr1", [D, D], F32, kind="ExternalInput")
    bsd = nc.dram_tensor("bs", [D, 8], F32, kind="ExternalInput")
    idnd = nc.dram_tensor("idn", [P, P], F32, kind="ExternalInput")
    yd = nc.dram_tensor("y", [NPC, D], F32, kind="ExternalOutput")

    xb = nc.dram_tensor("xb", [NPC, D], F32)
    xg = nc.dram_tensor("xg", [n_cores * NPC, D], F32, addr_space="Shared")
    sti = [nc.dram_tensor(f"sti{i}", [P, 2], F32) for i in range(2)]
    sto = [nc.dram_tensor(f"sto{i}", [P, 2], F32, addr_space="Shared")
           for i in range(2)]

    with TileContext(nc) as tc, ExitStack() as ctx:
        const = ctx.enter_context(tc.tile_pool(name="const", bufs=1))
        big = ctx.enter_context(tc.tile_pool(name="big", bufs=1))
        gpool = ctx.enter_context(tc.tile_pool(name="gp", bufs=6))
        spool = ctx.enter_context(tc.tile_pool(name="sp", bufs=6))
        small = ctx.enter_context(tc.tile_pool(name="sm", bufs=4))
        pagg = ctx.enter_context(tc.tile_pool(name="pagg", bufs=2, space="PSUM"))
        pmm = ctx.enter_context(tc.tile_pool(name="pmm", bufs=2, space="PSUM"))
        pres = ctx.enter_context(tc.tile_pool(name="pres", bufs=2, space="PSUM"))

        def ct(shape, dtype, srcap=None, name=None):
            t = const.tile(shape, dtype, tag=name)
            if srcap is not None:
                nc.sync.dma_start(out=t[:], in_=srcap)
            return t

        w0_t = ct([D, D], F32, w0d[:, :], "w0")
        wr0_t = ct([D, D], F32, wr0d[:, :], "wr0")
        w1_t = ct([D, D], F32, w1d[:, :], "w1")
        wr1_t = ct([D, D], F32, wr1d[:, :], "wr1")
        bias_t = ct([D, 8], F32, bsd[:, :], "bs")
        ident_t = ct([P, P], F32, idnd[:, :], "idn")
        oh_t = ct([P, totblk], I32, ohd[:, :], "oh")
        i16_t = ct([P, totcols], I16, i16d[:, :], "i16")
        iota_t = ct([P, P], I32, None, "iota")
        nc.gpsimd.iota(iota_t[:], pattern=[[1, P]], base=0, channel_multiplier=0)
        eps_t = ct([P, 1], F32, None, "eps")
        nc.vector.memset(eps_t[:], BN_EPS)

        xT = big.tile([P, NPC_PAD], F32, tag="xT")   # resident transposed acts

        scol = [ct([P, T_NODE], F32, None, f"scol{i}") for i in range(2)]
        qcol = [ct([P, T_NODE], F32, None, f"qcol{i}") for i in range(2)]

        qctr = [0]

        def layer(li, gsrc, w_t, wr_t, bcol, brcol, gcol, becol, out_dram):
            for t in range(T_NODE):
                gt = gpool.tile([P, tbmax, D], F32, tag="g")
                blk0 = sum(sum(nblk[tt]) for tt in range(t))
                boff = 0
                for b in range(N_BANKS):
                    nb = nblk[t][b]
                    ni = nb * P
                    c0 = (blk0 + boff) * 8
                    lo = b * bank_rows
                    hi = min(N, lo + bank_rows)
                    nc.gpsimd.dma_gather(
                        out_ap=gt[:, boff:boff + nb, :],
                        in_ap=gsrc[lo:hi, :],
                        idxs_ap=i16_t[:, c0:c0 + nb * 8],
                        num_idxs=ni,
                        num_idxs_reg=ni,
                        elem_size=D,
                        queue_num=qctr[0] % 4,
                    )
                    qctr[0] += 1
                    boff += nb
                tb = boff
                pa = pagg.tile([P, P], F32, tag="pa")
                for j in range(tb):
                    gj = blk0 + j
                    S = spool.tile([P, P], F32, tag="S")
                    nc.vector.tensor_tensor(
                        out=S[:],
                        in0=oh_t[:, gj:gj + 1].to_broadcast([P, P]),
                        in1=iota_t[:],
                        op=Alu.is_equal,
                    )
                    nc.tensor.matmul(pa[:], lhsT=gt[:, j, :], rhs=S[:],
                                     start=(j == 0), stop=(j == tb - 1))
                aggT = small.tile([P, P], F32, tag="aggT")
                nc.vector.tensor_copy(aggT[:], pa[:])
                pm = pmm.tile([P, P], F32, tag="pm")
                nc.tensor.matmul(pm[:], lhsT=w_t[:], rhs=aggT[:],
                                 start=True, stop=True)
                if "res" in ABLATE:
                    hT = None
                elif li == 0:
                    hrow = small.tile([P, P], F32, tag="hrow")
                    nc.sync.dma_start(out=hrow[:],
                                      in_=hs[t * P:(t + 1) * P, :])
                    ph = pres.tile([P, P], F32, tag="pq")
                    nc.tensor.transpose(ph[:], hrow[:], ident_t[:])
                    hTt = small.tile([P, P], F32, tag="hT")
                    nc.vector.tensor_copy(hTt[:], ph[:])
                    hT = hTt[:]
                else:
                    hT = xT[:, t * P:(t + 1) * P]
                newt = small.tile([P, P], F32, tag="newt")
                nc.scalar.activation(newt[:], pm[:], Act.Relu,
                                     bias=bias_t[:, bcol:bcol + 1])
                if hT is None:
                    rest = newt
                else:
                    pr = pres.tile([P, P], F32, tag="pq")
                    nc.tensor.matmul(pr[:], lhsT=wr_t[:], rhs=hT,
                                     start=True, stop=True)
                    rest = small.tile([P, P], F32, tag="rest")
                    nc.scalar.activation(rest[:], pr[:], Act.Relu,
                                         bias=bias_t[:, brcol:brcol + 1])
                ov = xT[:, t * P:(t + 1) * P]
                if "stats" in ABLATE:
                    nc.vector.tensor_tensor(out=ov, in0=newt[:], in1=rest[:],
                                            op=Alu.add)
                elif t == T_NODE - 1 and n_last < P:
                    nc.vector.scalar_tensor_tensor(
                        out=ov, in0=newt[:], scalar=0.0, in1=rest[:],
                        op0=Alu.add, op1=Alu.add)
                    nc.vector.memset(xT[:, t * P + n_last:(t + 1) * P], 0.0)
                    nc.vector.reduce_sum(out=scol[li][:, t:t + 1], in_=ov,
                                         axis=mybir.AxisListType.X)
                else:
                    nc.vector.scalar_tensor_tensor(
                        out=ov, in0=newt[:], scalar=0.0, in1=rest[:],
                        op0=Alu.add, op1=Alu.add,
                        accum_out=scol[li][:, t:t + 1])
                if "stats" not in ABLATE:
                    sq = small.tile([P, P], F32, tag="sq")
                    nc.scalar.activation(sq[:], ov, Act.Square,
                                         accum_out=qcol[li][:, t:t + 1])

            def store_phase():
                for t in range(T_NODE):
                    pt = pres.tile([P, P], F32, tag="pq")
                    nc.tensor.transpose(pt[:], xT[:, t * P:(t + 1) * P],
                                        ident_t[:])
                    stg2 = small.tile([P, P], F32, tag="stage")
                    nc.vector.tensor_copy(stg2[:], pt[:])
                    nrow = P if t < T_NODE - 1 else n_last
                    nc.sync.dma_start(out=out_dram[t * P:t * P + nrow, :],
                                      in_=stg2[:nrow, :])

            # --- global BN stats ---
            if "stats" in ABLATE:
                store_phase()
                return
            st_sb = small.tile([P, 2], F32, tag="stats")
            nc.vector.reduce_sum(out=st_sb[:, 0:1], in_=scol[li][:],
                                 axis=mybir.AxisListType.X)
            nc.vector.reduce_sum(out=st_sb[:, 1:2], in_=qcol[li][:],
                                 axis=mybir.AxisListType.X)
            nc.sync.dma_start(out=sti[li][:, :], in_=st_sb[:])
            if "ar" not in ABLATE:
                nc.gpsimd.collective_compute(
                    "AllReduce", Alu.add, replica_groups=groups,
                    ins=[sti[li].ap().opt()], outs=[sto[li].ap().opt()])
                stg = small.tile([P, 2], F32, tag="stg")
                nc.sync.dma_start(out=stg[:], in_=sto[li][:, :])
            else:
                stg = small.tile([P, 2], F32, tag="stg")
                nc.sync.dma_start(out=stg[:], in_=sti[li][:, :])
            mean = small.tile([P, 1], F32, tag="mean")
            nc.vector.tensor_scalar_mul(mean[:], stg[:, 0:1], 1.0 / N)
            ex2 = small.tile([P, 1], F32, tag="ex2")
            nc.vector.tensor_scalar_mul(ex2[:], stg[:, 1:2], 1.0 / N)
            var = small.tile([P, 1], F32, tag="var")
            nc.vector.tensor_tensor(out=var[:], in0=mean[:], in1=mean[:],
                                    op=Alu.mult)
            nc.vector.tensor_tensor(out=var[:], in0=ex2[:], in1=var[:],
                                    op=Alu.subtract)
            sd = small.tile([P, 1], F32, tag="sd")
            nc.scalar.activation(sd[:], var[:], Act.Sqrt, bias=eps_t[:, 0:1])
            rstd = small.tile([P, 1], F32, tag="rstd")
            nc.vector.reciprocal(rstd[:], sd[:])
            scale_t = small.tile([P, 1], F32, tag="scale")
            nc.vector.tensor_tensor(out=scale_t[:],
                                    in0=bias_t[:, gcol:gcol + 1],
                                    in1=rstd[:], op=Alu.mult)
            shift_t = small.tile([P, 1], F32, tag="shift")
            nc.vector.tensor_tensor(out=shift_t[:], in0=mean[:],
                                    in1=scale_t[:], op=Alu.mult)
            nc.vector.tensor_tensor(out=shift_t[:],
                                    in0=bias_t[:, becol:becol + 1],
                                    in1=shift_t[:], op=Alu.subtract)
            # BN apply in place on the resident transposed buffer
            nc.vector.tensor_scalar(
                out=xT[:, :], in0=xT[:, :],
                scalar1=scale_t[:, 0:1], scalar2=shift_t[:, 0:1],
                op0=Alu.mult, op1=Alu.add)
            store_phase()

        layer(0, hg, w0_t, wr0_t, 0, 1, 2, 3, xb)
        if "ag" not in ABLATE:
            nc.gpsimd.collective_compute(
                "AllGather", Alu.bypass, replica_groups=groups,
                ins=[xb.ap().opt()], outs=[xg.ap().opt()])
        if "l2" not in ABLATE:
            layer(1, xg, w1_t, wr1_t, 4, 5, 6, 7, yd)
        else:
            for t in range(T_NODE):
                z = small.tile([P, P], F32, tag="stage")
                nc.vector.memset(z[:], 0.0)
                nrow = P if t < T_NODE - 1 else n_last
                nc.sync.dma_start(out=yd[t * P:t * P + nrow, :],
                                  in_=z[:nrow, :])
    nc.compile()
    return nc


def _run(inputs, n_cores=N_CORES, trace=False, runner=None):
    h = np.asarray(inputs["h"], np.float32)
    src = np.asarray(inputs["src"])
    dst = np.asarray(inputs["dst"])
    N = h.shape[0]
    NPC = N // n_cores
    idx16_l, oh_l, nblk, meta = _preprocess(src, dst, N, n_cores)
    T_NODE = meta["T_NODE"]
    NPC_PAD = T_NODE * P

    nc = _build(N, nblk, n_cores)

    bs = np.stack([
        np.asarray(inputs["b0"], np.float32),
        np.asarray(inputs["br0"], np.float32),
        np.asarray(inputs["g0"], np.float32),
        np.asarray(inputs["be0"], np.float32),
        np.asarray(inputs["b1"], np.float32),
        np.asarray(inputs["br1"], np.float32),
        np.asarray(inputs["g1"], np.float32),
        np.asarray(inputs["be1"], np.float32),
    ], axis=1)
    idn = np.eye(P, dtype=np.float32)

    in_maps = []
    for c in range(n_cores):
        hs_c = np.zeros((NPC_PAD, D), np.float32)
        hs_c[:NPC] = h[c * NPC:(c + 1) * NPC]
        in_maps.append({
            "hg": h,
            "hs": hs_c,
            "i16": idx16_l[c],
            "oh": oh_l[c],
            "w0": np.asarray(inputs["W0"], np.float32),
            "wr0": np.asarray(inputs["Wr0"], np.float32),
            "w1": np.asarray(inputs["W1"], np.float32),
            "wr1": np.asarray(inputs["Wr1"], np.float32),
            "bs": bs,
            "idn": idn,
        })

    if runner is not None:
        results, extra = runner(nc, in_maps)
    else:
        res = bass_utils.run_bass_kernel_spmd(
            nc, in_maps, core_ids=list(range(n_cores)), trace=trace)
        results, extra = res.results, res

    xs = [results[c]["y"][:NPC] for c in range(n_cores)]
    out = np.concatenate(xs, axis=0)
    bsz = int(inputs["batch_size"])
    return out.reshape(bsz, -1, D).astype(np.float32), extra


def kernel(**inputs):
    out, _ = _run(inputs, trace=False)
    return out

